# revision 64
# baseline (speedup 1.0000x reference)
"""Trainium2 Bass kernel for CausalGNNLayer (per-node-type Linear, MoE-style routing).

Semantics (matching the reference):
    out[n, :] = x[n, :] @ W[node_types[n]].T + b[node_types[n]]
edge_index is unused by the op.

Strategy:
- Host-side routing-aware sharding: stable-sort nodes by type, split each
  type's node list into two halves -> 8 groups (4 types x 2 cores).  Each
  core runs a dense [P,512] x [512,512] per-type Linear in bf16.
- bf16 end-to-end I/O (x, W, out) halves HBM traffic vs fp32; rel err
  ~2.7e-3, far inside the 2e-2 gate.  PSUM accumulation stays fp32.
- Weight-stationary matmuls: lhsT = 128x128 W tile, rhs = a 512-node slab
  of x (moving).  Output lands transposed ([out_ch, nodes]) so BOTH the
  x loads and the out stores are 4KB-contiguous per partition -- 1KB
  descriptors measured only 254 GB/s on HW vs ~330 GB/s for 2KB+.
- PSUM drain (bias add + fp32->bf16) alternates between the Vector and
  Scalar engines so neither becomes the bottleneck.
- Host scatters the 8 output shards back into the full [N, 512] output.
"""

import numpy as np
import ml_dtypes
from contextlib import ExitStack

import concourse.bass as bass
import concourse.mybir as mybir
import concourse.tile as tile
from concourse.bass_utils import run_bass_kernel_spmd

N_CORES = 8
IN_CH = 512
OUT_CH = 512
NUM_TYPES = 4
P_BLK = 128          # SBUF partition count
KT = IN_CH // P_BLK  # 4 contraction tiles
NOC = OUT_CH // P_BLK  # 4 output-channel chunks
CB = 512             # moving-dim (node) slab per matmul / PSUM bank
S_MAIN = 2048        # nodes per superchunk (4 PSUM banks per o-phase)
XBUFS = 5            # x superchunk prefetch depth
OBUFS = 3            # output staging depth (each buf holds a full superchunk)
WARMUP_MM = 16       # garbage matmuls bridging the DMA-fill window (PE ramp)

# Set by test harness to capture HW profile; kernel works without it.
TRACE = False
LAST_RESULTS = None

_compile_cache: dict = {}

_legal_nop_counter = [0]


def _legalize_waits(nc: bass.Bass) -> None:
    """This walrus codegen only encodes ONE sync wait per engine instruction.
    Tile's scheduler attaches several.  Split: hoist all-but-one wait of any
    multi-wait instruction into preceding same-engine NoOps (one wait each) —
    semantically identical (the engine stalls on each wait in program order)."""
    for fn in nc.m.functions:
        for blk in fn.blocks:
            insts = blk.instructions
            out = []
            changed = False
            for inst in insts:
                si = inst.sync_info
                waits = list(si.on_wait) if si is not None and si.on_wait else []
                if len(waits) > 1:
                    changed = True
                    for w in waits[:-1]:
                        _legal_nop_counter[0] += 1
                        nop = mybir.InstNoOp(
                            name=f"waitsplit-{_legal_nop_counter[0]}",
                            ins=[],
                            outs=[],
                            engine=inst.engine,
                        )
                        nop.sync_info = mybir.SyncInfo(on_wait=[w], on_update=[])
                        out.append(nop)
                    inst.sync_info = mybir.SyncInfo(
                        on_wait=[waits[-1]], on_update=list(si.on_update or [])
                    )
                out.append(inst)
            if changed:
                blk.instructions = out


def _superchunks(P: int) -> list[tuple[int, int]]:
    """Split P into superchunks: small at both ends (shorter pipeline fill and
    drain around the big 2048-node steady-state chunks)."""
    assert P % 128 == 0
    sizes = []
    rem = P
    for s in (512, 512, 1024, 1024):  # ramp-up
        if rem >= s + 1024:
            sizes.append(s)
            rem -= s
    tail = []
    sub = rem % CB
    if sub:  # sub-512 remainder slab goes last
        tail.append(sub)
        rem -= sub
    if rem >= CB + S_MAIN:  # keep the final superchunk small
        if tail:
            tail.insert(0, CB)
            rem -= CB
        else:
            tail = [256, 256]
            rem -= CB
    while rem > 0:
        s = min(S_MAIN, rem)
        sizes.append(s)
        rem -= s
    sizes += tail
    out = []
    pos = 0
    for s in sizes:
        out.append((pos, s))
        pos += s
    assert pos == P
    return out


def _build_bass(P: int) -> bass.Bass:
    """One-core program: out[512, P] = (w_tiles . xT) + bias, weight-stationary."""
    nc = bass.Bass("TRN2")
    f32 = mybir.dt.float32
    bf = mybir.dt.bfloat16
    copy_fn = mybir.ActivationFunctionType.Identity

    xT = nc.dram_tensor("xT", [IN_CH, P], bf, kind="ExternalInput")
    xv = xT.ap().rearrange("(k p) n -> p k n", p=P_BLK)  # [128, KT, P]
    w = nc.dram_tensor("w", [P_BLK, NOC, KT, P_BLK], bf, kind="ExternalInput")
    bias = nc.dram_tensor("bias", [P_BLK, NOC], f32, kind="ExternalInput")
    out = nc.dram_tensor("out", [OUT_CH, P], bf, kind="ExternalOutput")
    out_v = out.ap().rearrange("(o p) n -> p o n", p=P_BLK)  # [128, NOC, P]

    sups = _superchunks(P)
    with ExitStack() as ctx:
        tc = ctx.enter_context(tile.TileContext(nc))
        wp = ctx.enter_context(tc.tile_pool(name="w", bufs=1))
        bp = ctx.enter_context(tc.tile_pool(name="b", bufs=1))
        mp = ctx.enter_context(tc.tile_pool(name="mm", bufs=1))
        xp = ctx.enter_context(tc.tile_pool(name="x", bufs=XBUFS))
        pp = ctx.enter_context(tc.tile_pool(name="ps", bufs=8, space="PSUM"))
        op = ctx.enter_context(tc.tile_pool(name="o", bufs=OBUFS))

        # x superchunk loads on the Sync queue; first superchunk issued FIRST
        # (DMA instructions serialize ~650ns each on their queue engine).
        x_tiles = []
        for si, (pos, S) in enumerate(sups[:XBUFS]):
            x_sb = xp.tile([P_BLK, KT, S_MAIN], bf, tag="x", name=f"x{si}")
            nc.sync.dma_start(x_sb[:, :, :S], xv[:, :, pos : pos + S])
            x_tiles.append(x_sb)

        # w / bias on the Scalar queue so they issue in parallel with x0.
        # Per-o loads (o-major layout, 1KB runs): the first o-phase only
        # needs w[:, 0] (128KB), and each later piece lands while the
        # previous o-phase computes -- no wait ever lands inside an
        # accumulation group (k loops stay within one piece).
        w_sb = wp.tile([P_BLK, NOC, KT, P_BLK], bf)
        for o in range(NOC):
            nc.scalar.dma_start(w_sb[:, o], w.ap()[:, o])
        b_sb = bp.tile([P_BLK, NOC], f32)
        nc.scalar.dma_start(b_sb[:], bias.ap())

        # Warmup matmuls on memset data: ramp the PE clock while DMAs fill.
        # Each uses a distinct rhs slice so none gets value-cache deduped.
        if WARMUP_MM:
            aw = mp.tile([P_BLK, P_BLK], bf, name="aw")
            xw = mp.tile([P_BLK, CB], bf, name="xw")
            nc.vector.memset(aw[:], 0.0)
            nc.vector.memset(xw[:], 0.0)
            pw = pp.tile([P_BLK, CB], f32, tag="ps", name="pw")
            for i in range(WARMUP_MM):
                nc.tensor.matmul(
                    pw[:, :384],
                    lhsT=aw[:],
                    rhs=xw[:, i * 8 : i * 8 + 384],
                    start=True,
                    stop=True,
                )

        for si, (pos, S) in enumerate(sups):
            nch = (S + CB - 1) // CB
            if si < XBUFS:
                x_sb = x_tiles[si]
            else:
                x_sb = xp.tile([P_BLK, KT, S_MAIN], bf, tag="x", name=f"x{si}")
                nc.sync.dma_start(x_sb[:, :, :S], xv[:, :, pos : pos + S])
            o_sb = op.tile([P_BLK, NOC, S_MAIN], bf, tag="o")
            for o in range(NOC):
                pss = [
                    pp.tile([P_BLK, CB], f32, tag="ps", name=f"ps{c}")
                    for c in range(nch)
                ]
                for c in range(nch):
                    cb = min(CB, S - c * CB)
                    for k in range(KT):
                        nc.tensor.matmul(
                            pss[c][:, :cb],
                            lhsT=w_sb[:, o, k, :],
                            rhs=x_sb[:, k, c * CB : c * CB + cb],
                            start=(k == 0),
                            stop=(k == KT - 1),
                        )
                for c in range(nch):
                    cb = min(CB, S - c * CB)
                    dst = o_sb[:, o, c * CB : c * CB + cb]
                    if (c + o) % 2 == 0:
                        nc.vector.tensor_scalar_add(
                            dst, pss[c][:, :cb], b_sb[:, o : o + 1]
                        )
                    else:
                        nc.scalar.activation(
                            dst, pss[c][:, :cb], copy_fn, bias=b_sb[:, o : o + 1]
                        )
            nc.sync.dma_start(out_v[:, :, pos : pos + S], o_sb[:, :, :S])
    _legalize_waits(nc)
    return nc


def _get_compiled(P: int) -> bass.Bass:
    if P not in _compile_cache:
        _compile_cache[P] = _build_bass(P)
    return _compile_cache[P]


def kernel(x, edge_index, node_types, W, b):
    global LAST_RESULTS
    x = np.asarray(x, dtype=np.float32)
    nt = np.asarray(node_types).astype(np.int64)
    W = np.asarray(W, dtype=np.float32)
    b = np.asarray(b, dtype=np.float32)
    N = x.shape[0]

    # Route nodes: stable sort by type, split each type across 2 cores.
    order = np.argsort(nt, kind="stable")
    counts = np.bincount(nt, minlength=NUM_TYPES)
    groups = []
    start = 0
    for t in range(NUM_TYPES):
        c = int(counts[t])
        idx = order[start : start + c]
        start += c
        h = (c + 1) // 2
        groups.append(idx[:h])
        groups.append(idx[h:])

    P = max(1, max(len(g) for g in groups))
    P = ((P + 127) // 128) * 128

    nc = _get_compiled(P)

    bf16 = ml_dtypes.bfloat16
    xb = x.astype(bf16)
    in_maps = []
    for gi, g in enumerate(groups):
        t = gi // 2
        xs = np.zeros((P, IN_CH), bf16)
        if len(g):
            xs[: len(g)] = xb[g]
        wT = W[t].T.astype(bf16)  # [in, out]
        in_maps.append(
            {
                "xT": np.ascontiguousarray(xs.T),
                "w": np.ascontiguousarray(
                    wT.reshape(KT, P_BLK, NOC, P_BLK).transpose(1, 2, 0, 3)
                ),
                "bias": np.ascontiguousarray(b[t].reshape(NOC, P_BLK).T),
            }
        )

    res = run_bass_kernel_spmd(nc, in_maps, list(range(N_CORES)), trace=TRACE)
    LAST_RESULTS = res

    out = np.empty((N, OUT_CH), np.float32)
    for gi, g in enumerate(groups):
        if len(g):
            out[g] = res.results[gi]["out"][:, : len(g)].T.astype(np.float32)
    return out
